# revision 3
# baseline (speedup 1.0000x reference)
"""Croston's method recurrence kernel for Trainium2 (Bass/Tile), 8-core SPMD.

Reference semantics (per series b, scanned over time t):
    nz  = x_t != 0
    Z_t = nz ? a*x_t + (1-a)*Z_{t-1} : Z_{t-1}
    V_t = nz ? a*q_{t-1} + (1-a)*V_{t-1} : V_{t-1}
    q_t = nz ? 1 : q_{t-1} + 1
    out_t = Z_t / V_t

Scaled reformulation (the factor a cancels in Z/V, so scan the a-free
recurrences; seeds are pre-divided by a on the host):
    s = sign(x) (= nz for x >= 0);  m = 1-s;  c = 1 - a*s
    Z'_t = c_t*Z'_{t-1} + x_t          seed Z0/a
    q_t  = m_t*q_{t-1} + 1             seed q0
    V'_t = c_t*V'_{t-1} + s_t*q_{t-1}  seed V0/a
    out  = Z'/V'

Each of the three recurrences maps onto TensorTensorScanArith (one
recurrence per partition, scan along the free dim). The scan instruction
exists on both the Vector (DVE) and GpSimd (Pool) engines; scan work is
split between them while the Scalar engine computes s/m/c and the DVE
computes e = s*shift(q), reciprocal and the final multiply.

Sharding: batch dim B=8192 split over 8 cores (1024 series each); each
core processes 8 partition-tiles of 128 series x T=2048 timesteps.
"""

import numpy as np
from contextlib import ExitStack

import concourse.bass as bass
import concourse.mybir as mybir
from concourse import tile
from concourse.bass_utils import run_bass_kernel_spmd

B, T = 8192, 2048
N_CORES = 8
B_SHARD = B // N_CORES       # 1024 series per core
P = 128                      # SBUF partitions
N_TILES = B_SHARD // P       # 8 row-tiles per core

_DT = mybir.dt.float32
_OP = mybir.AluOpType
_ACT = mybir.ActivationFunctionType

TRACE = False                # set by test harness to capture a HW profile
LAST_RESULTS = None          # BassKernelResults of the last run (for test.py)

# TensorTensorScanArith only exists on the DVE; walrus rejects it on Pool
# ("Instruction engine check failed"), and the custom-DVE reciprocal ops hit
# "ISA wrong length" in this compiler build.  So: DVE runs exactly the three
# scans, GpSimd (Pool) runs the two tensor-tensor multiplies, and the Scalar
# engine computes sign/coefficients and the reciprocal as Exp(-Ln(V)).

_nc_cache: dict[tuple, object] = {}


def _split_tsp_waits(nc):
    """walrus's S2S2D2_STT codegen template ("Too many sync wait commands",
    CoreV2GenImpl.cpp setupSyncWait) accepts at most one embedded sync wait
    per TensorScalarPtr instruction. Hoist every wait of a multi-wait
    TensorScalarPtr onto single-wait NoOps inserted immediately before it
    in the same engine queue (engines run their queue in order, so the
    waits still gate the instruction)."""
    skip = (mybir.InstNoOp,)
    # Custom-DVE / raw-ISA instructions cannot carry ANY embedded wait or
    # update (walrus "ISA wrong length"); everything else tolerates exactly
    # one wait.
    zero_wait = (mybir.InstCustomDveAnt, mybir.InstISA)
    for fn in nc.m.functions:
        for blk in fn.blocks:
            out = []
            for inst in blk.instructions:
                si = inst.sync_info
                if si is None or isinstance(inst, skip):
                    out.append(inst)
                    continue
                is_zero = isinstance(inst, zero_wait)
                if len(si.on_wait) > (0 if is_zero else 1):
                    for k, w in enumerate(si.on_wait):
                        nop = mybir.InstNoOp(name=f"{inst.name}-w{k}")
                        nop.engine = inst.engine
                        nop.sync_info = mybir.SyncInfo(on_wait=[w], on_update=[])
                        out.append(nop)
                    si = mybir.SyncInfo(on_wait=[], on_update=si.on_update)
                    inst.sync_info = si
                out.append(inst)
                if is_zero and len(si.on_update) > 0:
                    # engines run their queue in order, so an update on a
                    # trailing NoOp fires only after the instruction retires
                    nop = mybir.InstNoOp(name=f"{inst.name}-u")
                    nop.engine = inst.engine
                    nop.sync_info = mybir.SyncInfo(
                        on_wait=[], on_update=si.on_update
                    )
                    inst.sync_info = mybir.SyncInfo(on_wait=[], on_update=[])
                    out.append(nop)
            blk.instructions = out
    return nc


def _build_nc(a: float, split_waits: bool = True):
    a = float(np.float32(a))

    nc = bass.Bass()
    x = nc.dram_tensor("x", [B_SHARD, T], _DT, kind="ExternalInput")
    z0 = nc.dram_tensor("z0", [B_SHARD, 1], _DT, kind="ExternalInput")
    v0 = nc.dram_tensor("v0", [B_SHARD, 1], _DT, kind="ExternalInput")
    q0 = nc.dram_tensor("q0", [B_SHARD, 1], _DT, kind="ExternalInput")
    out = nc.dram_tensor("out", [B_SHARD, T], _DT, kind="ExternalOutput")

    xv = x[:].rearrange("(n p) t -> n p t", p=P)
    ov = out[:].rearrange("(n p) t -> n p t", p=P)
    # State vectors packed as one (128, N_TILES) SBUF tile: column i holds
    # the 128 per-series init values of row-tile i.
    z0v = z0[:].rearrange("(n p) o -> p (n o)", p=P)
    v0v = v0[:].rearrange("(n p) o -> p (n o)", p=P)
    q0v = q0[:].rearrange("(n p) o -> p (n o)", p=P)

    with tile.TileContext(nc) as tc:
        with ExitStack() as ctx:
            const = ctx.enter_context(tc.tile_pool(name="const", bufs=1))
            ones = const.tile([P, T], _DT, tag="ones")
            nc.gpsimd.memset(ones[:], 1.0)
            z0s = const.tile([P, N_TILES], _DT, tag="z0s")
            v0s = const.tile([P, N_TILES], _DT, tag="v0s")
            q0s = const.tile([P, N_TILES], _DT, tag="q0s")
            nc.sync.dma_start(z0s[:], z0v)
            nc.sync.dma_start(v0s[:], v0v)
            nc.sync.dma_start(q0s[:], q0v)

            xp = ctx.enter_context(tc.tile_pool(name="xp", bufs=2))
            op = ctx.enter_context(tc.tile_pool(name="op", bufs=2))
            wp = ctx.enter_context(tc.tile_pool(name="wp", bufs=2))

            for i in range(N_TILES):
                xt = xp.tile([P, T], _DT, tag="x")
                nc.sync.dma_start(xt[:], xv[i])

                zprev = z0s[:, i : i + 1]
                vprev = v0s[:, i : i + 1]
                qprev = q0s[:, i : i + 1]

                # s = sign(x) (x >= 0), m = 1-s, c = 1-a*s on Scalar
                s = wp.tile([P, T], _DT, tag="s")
                nc.scalar.activation(s[:], xt[:], _ACT.Sign)
                m = wp.tile([P, T], _DT, tag="m")
                nc.scalar.activation(m[:], s[:], _ACT.Copy, bias=1.0, scale=-1.0)
                c = wp.tile([P, T], _DT, tag="c")
                nc.scalar.activation(c[:], s[:], _ACT.Copy, bias=1.0, scale=-a)

                # q_t = m_t*q_{t-1} + 1
                q = wp.tile([P, T], _DT, tag="q")
                nc.vector.tensor_tensor_scan(
                    q[:], m[:], ones[:], qprev, _OP.mult, _OP.add
                )

                # e = s * q_{t-1} (shifted q, seeded with carry).  On the
                # DVE: keeps the q -> e -> V chain on one engine queue (no
                # cross-engine semaphore round-trip) and a DVE TT costs only
                # ~0.6 ns/col vs ~2.7 on Pool.
                e = wp.tile([P, T], _DT, tag="e")
                nc.vector.tensor_mul(e[:, 0:1], s[:, 0:1], qprev)
                nc.vector.tensor_mul(e[:, 1:], s[:, 1:], q[:, : T - 1])

                # Z'_t = c_t*Z'_{t-1} + x_t
                Z = wp.tile([P, T], _DT, tag="Z")
                nc.vector.tensor_tensor_scan(
                    Z[:], c[:], xt[:], zprev, _OP.mult, _OP.add
                )

                # V'_t = c_t*V'_{t-1} + e_t
                V = wp.tile([P, T], _DT, tag="V")
                nc.vector.tensor_tensor_scan(
                    V[:], c[:], e[:], vprev, _OP.mult, _OP.add
                )

                # r = 1/V as Exp(-Ln(V)) on the Scalar engine (the DVE
                # InstReciprocal costs 6.3 ns/col; these two passes cost
                # 2 x 0.98 ns/col on an otherwise-idle engine)
                lnv = wp.tile([P, T], _DT, tag="lnv")
                nc.scalar.activation(lnv[:], V[:], _ACT.Ln)
                r = wp.tile([P, T], _DT, tag="r")
                nc.scalar.activation(r[:], lnv[:], _ACT.Exp, bias=0.0, scale=-1.0)

                ot = op.tile([P, T], _DT, tag="o")
                nc.gpsimd.tensor_mul(ot[:], Z[:], r[:])

                nc.sync.dma_start(ov[i], ot[:])
    if split_waits:
        _split_tsp_waits(nc)
    return nc


def _get_nc(a: float):
    key = int(np.float32(a).view(np.int32))
    nc = _nc_cache.get(key)
    if nc is None:
        nc = _build_nc(a)
        _nc_cache[key] = nc
    return nc


def kernel(x, alpha, Z0, V0, q0):
    global LAST_RESULTS
    x = np.ascontiguousarray(np.asarray(x, dtype=np.float32))
    a = float(np.asarray(alpha, dtype=np.float32).reshape(-1)[0])
    a_eff = np.float32(max(a, 1e-20))
    # the a factor cancels in Z/V; pre-divide the Z/V seeds by a on host
    Z0 = (np.asarray(Z0, dtype=np.float32) / a_eff).reshape(B, 1)
    V0 = (np.asarray(V0, dtype=np.float32) / a_eff).reshape(B, 1)
    q0 = np.asarray(q0, dtype=np.float32).reshape(B, 1)

    nc = _get_nc(a)
    in_maps = []
    for k in range(N_CORES):
        s = slice(k * B_SHARD, (k + 1) * B_SHARD)
        in_maps.append(
            {
                "x": x[s],
                "z0": np.ascontiguousarray(Z0[s]),
                "v0": np.ascontiguousarray(V0[s]),
                "q0": np.ascontiguousarray(q0[s]),
            }
        )

    res = run_bass_kernel_spmd(nc, in_maps, list(range(N_CORES)), trace=TRACE)
    LAST_RESULTS = res
    return np.concatenate([res.results[k]["out"] for k in range(N_CORES)], axis=0)
